# revision 29
# baseline (speedup 1.0000x reference)
"""Trainium2 Bass kernel for nn_Attention_41704132444382.

Masked-linear QKV projection + 16-head attention + masked-linear output
projection, tensor-parallel over heads across 8 NeuronCores (2 heads/core).

v3: fully fused single-loop design. The PE clock on TRN2 ramps to 2.4GHz
only under sustained back-to-back work and drops to 1.2GHz whenever the
queue gaps; a bare attention loop is exp-activation-bound with ~0.4us PE
idle per key-tile, which pins the clock at half speed. So everything that
is not the S->exp->PV chain is turned into schedulable PE filler injected
into specific key-tile slots:

  - QKV projection chains for batch 1 run inside batch 0's attention.
  - V transposes, out-projection tiles, and the softmax-normalization
    transposes are likewise spread into exp-bound stretches.
  - Scores psum tiles are [128,1024] (one exp per head per key-tile);
    PV runs one key-tile behind S so the in-order PE queue never parks.
  - Normalization: colsum row -> 16 tiny PE transposes -> reciprocal on
    [128,16] (partition-major: ~100ns vs 6.4us row-major) -> PE transpose
    back -> partition_broadcast -> one DVE mult. All emitted lazily one
    block later; psum accumulators are freed by an immediate [65,1024]
    SBUF evacuation.
  - QKV stays float32r (precision); S/PV/out-proj in bf16 (~7e-3 rel err
    vs the 2e-2 gate). Host gates the masked weights and sums bf16
    partial outputs; f32r dram params let hwdge queues DMA without casts.
"""

import os
import sys

import numpy as np

sys.path.insert(0, "/opt/trn_rl_repo")

import concourse.bass as bass
import concourse.mybir as mybir
from concourse import bacc
from concourse.masks import make_identity
from concourse.tile import TileContext

DIM = 1024
HEADS = 16
B = 2
N = 2048
T = B * N  # 4096 flattened tokens
NCORES = 8
HPC = HEADS // NCORES  # 2 heads per core
DV = HPC * 64  # 128 head-dims per core
SCALE = DIM ** (-0.5)  # 1/32

F32 = mybir.dt.float32
F32R = mybir.dt.float32r
BF16 = mybir.dt.bfloat16


def build_nc():
    nc = bacc.Bacc("TRN2", target_bir_lowering=True)
    # f32r dram params share bytes with f32 but let hwdge queues DMA them
    xT_d = nc.declare_dram_parameter("xT", [DIM, T], F32R, isOutput=False)
    wqkvT_d = nc.declare_dram_parameter("wqkvT", [DIM, 384], F32R, isOutput=False)
    woT_d = nc.declare_dram_parameter("woT", [DV, DIM], F32, isOutput=False)
    out_d = nc.declare_dram_parameter("out", [T, DIM], BF16, isOutput=True)

    mult = mybir.AluOpType.mult
    Exp = mybir.ActivationFunctionType.Exp

    with TileContext(nc) as tc:
        with (
            tc.tile_pool(name="persist", bufs=1) as pp,
            tc.tile_pool(name="xq", bufs=22) as xp,
            tc.tile_pool(name="es", bufs=4) as ep,
            tc.tile_pool(name="cs", bufs=2) as csp,
            tc.tile_pool(name="bc", bufs=1) as bcp,
            tc.tile_pool(name="ob", bufs=3) as obp,
            tc.tile_pool(name="s_ps", bufs=2, space="PSUM") as sps,
            tc.tile_pool(name="pv_ps", bufs=2, space="PSUM") as pvps,
        ):
            wqkv_g = pp.tile([128, 8 * 384], F32R)  # [k-part, (kt, o)]
            wo_g = pp.tile([128, 1024], BF16)
            qT = pp.tile([128, 4096], BF16)
            kTt = pp.tile([128, 4096], BF16)
            vT = pp.tile([128, 4096], BF16)
            # V with ones column: [t-part, (h, jt32, dv64|1)]
            vv = pp.tile([128, 2 * 32 * 65], BF16)
            attnT = [pp.tile([128, 2048], BF16, name=f"attnT{bb}") for bb in range(B)]
            identf = pp.tile([128, 128], F32)
            identb = pp.tile([128, 128], BF16)

            make_identity(nc, identf[:])
            ones1r = pp.tile([1, 64], F32R)
            nc.vector.tensor_copy(identb[:], identf[:])
            ones_f = pp.tile([128, 64], F32)
            nc.vector.memset(ones_f[:], 1.0)
            nc.vector.tensor_copy(ones1r[:], ones_f[0:1, :])
            vv_v = vv[:].rearrange("p (v j c) -> p v j c", v=2, c=65)
            nc.vector.tensor_copy(
                vv_v[:, :, :, 64:65],
                ones_f[:].rearrange("p (v j c) -> p v j c", v=2, c=1),
            )

            # ---------- upfront DMAs ----------
            # wqkv per-kt so the first chain starts as soon as kt=0 lands
            for kt in range(8):
                nc.sync.dma_start(
                    wqkv_g[:, kt * 384 : (kt + 1) * 384],
                    wqkvT_d[kt * 128 : (kt + 1) * 128, :],
                )
            wor = pp.tile([128, 1024], F32)
            nc.sync.dma_start(wor[:], woT_d[:])
            nc.vector.tensor_copy(wo_g[:], wor[:])

            xqs = []
            for q in range(4):
                xq = [
                    xp.tile([128, 1024], F32R, tag="xq", name=f"xq{q}_{i}")
                    for i in range(8)
                ]
                for kt in range(8):
                    nc.gpsimd.dma_start(
                        xq[kt][:],
                        xT_d[kt * 128 : (kt + 1) * 128, q * 1024 : (q + 1) * 1024],
                    )
                xqs.append(xq)

            # ---------- PE warmup: identity transposes with no DMA deps ----------
            for w in range(24):
                wt = sps.tile([128, 128], F32, tag="s", name=f"warm{w}")
                nc.tensor.transpose(wt[:], identf[:], identf[:])

            # ---------- emit helpers ----------
            evac_flip = [0]

            chain_parts = {}

            def emit_chain(q, ot, th, eng=None, part=None):
                """QKV projection chain: 512 tokens x 128 out-channels.

                part=0 emits the first 4 accumulation matmuls, part=1 the
                last 4 plus the evacuation (same psum tile across both);
                part=None emits the whole chain."""
                dest = (qT, kTt, vT)[ot]
                if part in (None, 0):
                    ps = sps.tile([128, 512], F32, tag="s", name=f"ch{q}_{ot}_{th}")
                    chain_parts[(q, ot, th)] = ps
                else:
                    ps = chain_parts.pop((q, ot, th))
                kts = range(8) if part is None else (
                    range(4) if part == 0 else range(4, 8)
                )
                for kt in kts:
                    nc.tensor.matmul(
                        ps[:],
                        wqkv_g[:, kt * 384 + ot * 128 : kt * 384 + (ot + 1) * 128],
                        xqs[q][kt][:, th * 512 : (th + 1) * 512],
                        start=(kt == 0),
                        stop=(kt == 7),
                    )
                if part == 0:
                    return
                col = q * 1024 + th * 512
                if eng is None:
                    eng = nc.vector if evac_flip[0] % 2 == 0 else nc.scalar
                    evac_flip[0] += 1
                if eng is nc.scalar:
                    nc.scalar.copy(dest[:, col : col + 512], ps[:])
                else:
                    eng.tensor_copy(dest[:, col : col + 512], ps[:])

            def emit_vtrans(jt):
                ptv = sps.tile([128, 128], BF16, tag="s", name=f"ptv{jt}")
                nc.tensor.transpose(ptv[:], vT[:, jt * 128 : (jt + 1) * 128], identb[:])
                nc.vector.tensor_copy(
                    vv_v[:, :, jt, 0:64],
                    ptv[:].rearrange("p (v c) -> p v c", v=2),
                )

            ob_flip = [0]

            def emit_po(pb, pib, tt):
                tg = pib * 8 + tt
                po = sps.tile([128, 1024], F32, tag="s", name=f"po{pb}_{pib}_{tt}")
                for oh in range(2):
                    nc.tensor.matmul(
                        po[:, oh * 512 : (oh + 1) * 512],
                        attnT[pb][:, tg * 128 : (tg + 1) * 128],
                        wo_g[:, oh * 512 : (oh + 1) * 512],
                        start=True,
                        stop=True,
                    )
                ob = obp.tile([128, 1024], BF16, tag="ob", name=f"ob{pb}_{pib}_{tt}")
                nc.vector.tensor_copy(ob[:], po[:])
                row = pb * 2048 + tg * 128
                nc.sync.dma_start(out_d[row : row + 128, :], ob[:])

            norm_state = {}

            def emit_pvs(b, ib, pv):
                """Evacuate the pv accumulators (fast, frees psum banks).
                The colsum row goes to a partition-0 tile so the norm's PE
                transposes can read it."""
                pvs = []
                for h in range(2):
                    t = csp.tile([64, 1024], F32, tag="pvs", name=f"pvs{b}_{ib}_{h}")
                    nc.vector.tensor_copy(t[:], pv[h][0:64, :])
                    cs = csp.tile([1, 1024], F32, tag="cs", name=f"cs{b}_{ib}_{h}")
                    nc.vector.tensor_copy(cs[:], pv[h][64:65, :])
                    pvs.append((t, cs))
                norm_state[(b, ib)] = pvs

            def emit_norm1(b, ib):
                """Norm stage 1: colsum rows -> partition-major -> reciprocal."""
                pvs = norm_state[(b, ib)]
                ptp = sps.tile([128, 16], F32, tag="s", name=f"ptp{b}_{ib}")
                for h in range(2):
                    for blk in range(8):
                        c = blk * 2 + h
                        nc.tensor.transpose(
                            ptp[:, c : c + 1],
                            pvs[h][1][0:1, blk * 128 : (blk + 1) * 128],
                            identf[0:1, 0:1],
                        )
                rt = csp.tile([128, 16], F32, tag="rt", name=f"rt{b}_{ib}")
                nc.vector.tensor_copy(rt[:], ptp[:])
                rcp = csp.tile([128, 16], F32, tag="rcp", name=f"rcp{b}_{ib}")
                nc.vector.reciprocal(rcp[:], rt[:])
                norm_state[(b, ib, "rcp")] = rcp

            def emit_norm2(b, ib, pe_bcast=False):
                """Norm stage 2: transpose back, broadcast, apply."""
                pvs = norm_state.pop((b, ib))
                rcp = norm_state.pop((b, ib, "rcp"))
                r2s = []
                for h in range(2):
                    r2p = sps.tile([1, 1024], F32, tag="s", name=f"r2p{b}_{ib}_{h}")
                    for blk in range(8):
                        c = blk * 2 + h
                        nc.tensor.transpose(
                            r2p[0:1, blk * 128 : (blk + 1) * 128],
                            rcp[:, c : c + 1],
                            identf[:],
                        )
                    r2 = csp.tile([1, 1024], F32, tag="r2", name=f"r2{b}_{ib}_{h}")
                    nc.vector.tensor_copy(r2[:], r2p[:])
                    r2s.append(r2)
                for h in range(2):
                    if pe_bcast:
                        r2r = csp.tile(
                            [1, 1024], F32R, tag="r2r", name=f"r2r{b}_{ib}_{h}"
                        )
                        nc.vector.tensor_copy(r2r[:], r2s[h][:])
                        bcp_ps = sps.tile(
                            [64, 1024], F32, tag="s", name=f"bcp{b}_{ib}_{h}"
                        )
                        for ih in range(2):
                            nc.tensor.matmul(
                                bcp_ps[:, ih * 512 : (ih + 1) * 512],
                                ones1r[:],
                                r2r[0:1, ih * 512 : (ih + 1) * 512],
                                start=True,
                                stop=True,
                            )
                        nc.vector.tensor_tensor(
                            attnT[b][
                                h * 64 : (h + 1) * 64, ib * 1024 : (ib + 1) * 1024
                            ],
                            pvs[h][0][:],
                            bcp_ps[:],
                            mult,
                        )
                    else:
                        bc = bcp.tile([64, 1024], F32, tag="bc", name=f"bc{b}_{ib}_{h}")
                        nc.gpsimd.partition_broadcast(bc[:], r2s[h][:])
                        nc.vector.tensor_tensor(
                            attnT[b][
                                h * 64 : (h + 1) * 64, ib * 1024 : (ib + 1) * 1024
                            ],
                            pvs[h][0][:],
                            bc[:],
                            mult,
                        )

            def emit_norm(b, ib):
                emit_norm1(b, ib)
                emit_norm2(b, ib)

            # ---------- bootstrap: QKV(b0) + V(b0) transposes ----------
            for q in (0, 1):
                for ot in (1, 2, 0):  # K, V, Q order
                    for th in range(2):
                        emit_chain(q, ot, th)
            for jt in range(2):
                emit_vtrans(jt)

            # ---------- fused attention loop ----------
            # filler[block][jt] = list of callables emitted after that jt
            filler = {bi: {} for bi in range(4)}
            # block 0: QKV chains for batch 1 (K and V; Q deferred)
            b0_sched = [
                (1, 2, 0), (2, 2, 0), (1, 2, 1), (2, 2, 1),
                (1, 3, 0), (2, 3, 0), (1, 3, 1), (2, 3, 1),
            ]
            for i, (ot, q, th) in enumerate(b0_sched):
                jt = 2 * i  # halves at (jt, jt+1) covering all 16 jts
                filler[0].setdefault(jt, []).append(
                    lambda q=q, ot=ot, th=th: emit_chain(q, ot, th, part=0)
                )
                filler[0].setdefault(jt + 1, []).append(
                    lambda q=q, ot=ot, th=th: emit_chain(
                        q, ot, th, eng=nc.vector, part=1
                    )
                )
            for i in range(14):  # vtrans 2..15 spread through block 0
                filler[0].setdefault(i, []).append(lambda jt=2 + i: emit_vtrans(jt))
            # block 1: Q(b1,ib0), V(b1) transposes, norm(block0), po(b0,ib0)
            filler[1].setdefault(0, []).append(lambda: emit_chain(2, 0, 0, part=0))
            filler[1].setdefault(1, []).append(
                lambda: emit_chain(2, 0, 0, eng=nc.vector, part=1)
            )
            filler[1].setdefault(2, []).append(lambda: emit_chain(2, 0, 1, part=0))
            filler[1].setdefault(3, []).append(
                lambda: emit_chain(2, 0, 1, eng=nc.vector, part=1)
            )
            for i in range(16):
                filler[1].setdefault(2 + i // 4, []).append(
                    lambda jt=16 + i: emit_vtrans(jt)
                )
            filler[1].setdefault(6, []).append(lambda: emit_norm1(0, 0))
            filler[1].setdefault(7, []).append(lambda: emit_norm2(0, 0))
            for tt in range(8):
                filler[1].setdefault(8 + tt, []).append(
                    lambda tt=tt: emit_po(0, 0, tt)
                )
            # block 2: Q(b1,ib1), norm(block1), po(b0,ib1)
            filler[2].setdefault(0, []).append(lambda: emit_chain(3, 0, 0, part=0))
            filler[2].setdefault(1, []).append(
                lambda: emit_chain(3, 0, 0, eng=nc.vector, part=1)
            )
            filler[2].setdefault(2, []).append(lambda: emit_chain(3, 0, 1, part=0))
            filler[2].setdefault(3, []).append(
                lambda: emit_chain(3, 0, 1, eng=nc.vector, part=1)
            )
            filler[2].setdefault(3, []).append(lambda: emit_norm1(0, 1))
            filler[2].setdefault(4, []).append(lambda: emit_norm2(0, 1))
            for tt in range(8):
                filler[2].setdefault(8 + tt, []).append(
                    lambda tt=tt: emit_po(0, 1, tt)
                )
            # block 3: norm(block2), po(b1,ib0) (4 units held for the tail)
            filler[3].setdefault(1, []).append(lambda: emit_norm1(1, 0))
            filler[3].setdefault(2, []).append(lambda: emit_norm2(1, 0))
            for tt in range(4):
                filler[3].setdefault(5 + 2 * tt, []).append(
                    lambda tt=tt: emit_po(1, 0, tt)
                )

            for bi in range(4):
                b, ib = bi // 2, bi % 2
                i0 = b * 2048 + ib * 1024
                pv = [
                    pvps.tile([65, 1024], F32, tag="pv", name=f"pv{b}_{ib}_{h}")
                    for h in range(2)
                ]
                prev = None
                for jt in range(16):
                    j0 = b * 2048 + jt * 128
                    e_h = [
                        ep.tile([128, 1024], BF16, tag="e", name=f"e{b}_{ib}_{jt}_{h}")
                        for h in range(2)
                    ]
                    for h in range(2):
                        st = sps.tile(
                            [128, 1024], F32, tag="s", name=f"s{b}_{ib}_{jt}_{h}"
                        )
                        kTl = kTt[h * 64 : (h + 1) * 64, j0 : j0 + 128]
                        for ih in range(2):
                            nc.tensor.matmul(
                                st[:, ih * 512 : (ih + 1) * 512],
                                kTl,
                                qT[
                                    h * 64 : (h + 1) * 64,
                                    i0 + ih * 512 : i0 + (ih + 1) * 512,
                                ],
                                start=True,
                                stop=True,
                                tile_position=(h * 64, 0),
                            )
                        nc.scalar.activation(e_h[h][:], st[:], Exp, scale=SCALE)
                    if prev is not None:
                        jv = b * 16 + (jt - 1)
                        for h in range(2):
                            for ih in range(2):
                                nc.tensor.matmul(
                                    pv[h][:, ih * 512 : (ih + 1) * 512],
                                    vv_v[:, h, jv, :],
                                    prev[h][:, ih * 512 : (ih + 1) * 512],
                                    start=(jt - 1 == 0),
                                    stop=False,
                                )
                    prev = e_h
                    for fn in filler[bi].get(jt, ()):
                        fn()
                # flush PV for jt=15
                jv = b * 16 + 15
                for h in range(2):
                    for ih in range(2):
                        nc.tensor.matmul(
                            pv[h][:, ih * 512 : (ih + 1) * 512],
                            vv_v[:, h, jv, :],
                            prev[h][:, ih * 512 : (ih + 1) * 512],
                            start=False,
                            stop=True,
                        )
                emit_pvs(b, ib, pv)

            # ---------- tail: last norm overlapped with reserved po units ----------
            emit_po(1, 0, 4)
            emit_po(1, 0, 5)
            emit_norm1(1, 1)
            emit_po(1, 0, 6)
            emit_norm2(1, 1, pe_bcast=True)
            emit_po(1, 0, 7)
            for tt in range(8):
                emit_po(1, 1, tt)

    nc.compile()
    return nc


_NC = None


def _get_nc():
    global _NC
    if _NC is None:
        _NC = build_nc()
    return _NC


def _gate(mask):
    """Exact jax fp32 gate: sigmoid(m) > 0.5 (fp32 logistic rounding)."""
    mask = np.asarray(mask, dtype=np.float32)
    return (np.float32(1.0) / (np.float32(1.0) + np.exp(-mask))) > np.float32(0.5)


def make_in_maps(x, qkv_weight, qkv_weight_mask, out_weight, out_weight_mask):
    x = np.asarray(x, dtype=np.float32)
    wq = np.asarray(qkv_weight, dtype=np.float32) * _gate(qkv_weight_mask)
    wo = np.asarray(out_weight, dtype=np.float32) * _gate(out_weight_mask)

    xT = np.ascontiguousarray(x.reshape(T, DIM).T)
    in_maps = []
    for c in range(NCORES):
        r0 = c * DV
        sl = slice(r0, r0 + DV)
        w_shard = np.concatenate(
            [wq[sl], wq[DIM + r0 : DIM + r0 + DV], wq[2 * DIM + r0 : 2 * DIM + r0 + DV]],
            axis=0,
        )  # [384, 1024] rows = (q h1,h2 | k h1,h2 | v h1,h2)
        in_maps.append(
            {
                "xT": xT,
                "wqkvT": np.ascontiguousarray(w_shard.T),
                "woT": np.ascontiguousarray(wo[:, sl].T),
            }
        )
    return in_maps


LAST_RESULTS = None  # BassKernelResults of the most recent run (for profiling)


def kernel(
    x,
    qkv_weight,
    qkv_weight_mask,
    out_weight,
    out_weight_mask,
    out_bias,
    out_bias_mask,
    _trace=False,
    _tmpdir=None,
):
    global LAST_RESULTS
    from concourse.bass_utils import run_bass_kernel_spmd

    nc = _get_nc()
    in_maps = make_in_maps(x, qkv_weight, qkv_weight_mask, out_weight, out_weight_mask)
    res = run_bass_kernel_spmd(
        nc, in_maps, list(range(NCORES)), trace=_trace, tmpdir=_tmpdir
    )
    LAST_RESULTS = res
    out = np.zeros((T, DIM), dtype=np.float32)
    for r in res.results:
        out += np.asarray(r["out"]).astype(np.float32)
    out_bias = np.asarray(out_bias, dtype=np.float32)
    out += np.where(_gate(out_bias_mask), out_bias, np.float32(0.0))[None, :]
    return out.reshape(B, N, DIM)


# revision 35
# speedup vs baseline: 1.0042x; 1.0042x over previous
"""Trainium2 Bass kernel for nn_Attention_41704132444382.

Masked-linear QKV projection + 16-head attention + masked-linear output
projection, tensor-parallel over heads across 8 NeuronCores (2 heads/core).

v3: fully fused single-loop design. The PE clock on TRN2 ramps to 2.4GHz
only under sustained back-to-back work and drops to 1.2GHz whenever the
queue gaps; a bare attention loop is exp-activation-bound with ~0.4us PE
idle per key-tile, which pins the clock at half speed. So everything that
is not the S->exp->PV chain is turned into schedulable PE filler injected
into specific key-tile slots:

  - QKV projection chains for batch 1 run inside batch 0's attention.
  - V transposes, out-projection tiles, and the softmax-normalization
    transposes are likewise spread into exp-bound stretches.
  - Scores psum tiles are [128,1024] (one exp per head per key-tile);
    PV runs one key-tile behind S so the in-order PE queue never parks.
  - Normalization: colsum row -> 16 tiny PE transposes -> reciprocal on
    [128,16] (partition-major: ~100ns vs 6.4us row-major) -> PE transpose
    back -> partition_broadcast -> one DVE mult. All emitted lazily one
    block later; psum accumulators are freed by an immediate [65,1024]
    SBUF evacuation.
  - QKV stays float32r (precision); S/PV/out-proj in bf16 (~7e-3 rel err
    vs the 2e-2 gate). Host gates the masked weights and sums bf16
    partial outputs; f32r dram params let hwdge queues DMA without casts.
"""

import os
import sys

import numpy as np

sys.path.insert(0, "/opt/trn_rl_repo")

import concourse.bass as bass
import concourse.mybir as mybir
from concourse import bacc
from concourse.masks import make_identity
from concourse.tile import TileContext

DIM = 1024
HEADS = 16
B = 2
N = 2048
T = B * N  # 4096 flattened tokens
NCORES = 8
HPC = HEADS // NCORES  # 2 heads per core
DV = HPC * 64  # 128 head-dims per core
SCALE = DIM ** (-0.5)  # 1/32

F32 = mybir.dt.float32
F32R = mybir.dt.float32r
BF16 = mybir.dt.bfloat16


def build_nc():
    nc = bacc.Bacc("TRN2", target_bir_lowering=True)
    # f32r dram params share bytes with f32 but let hwdge queues DMA them
    xT_d = nc.declare_dram_parameter("xT", [DIM, T], F32R, isOutput=False)
    wqkvT_d = nc.declare_dram_parameter("wqkvT", [DIM, 384], F32R, isOutput=False)
    woT_d = nc.declare_dram_parameter("woT", [DV, DIM], F32, isOutput=False)
    out_d = nc.declare_dram_parameter("out", [T, DIM], BF16, isOutput=True)

    mult = mybir.AluOpType.mult
    Exp = mybir.ActivationFunctionType.Exp

    with TileContext(nc) as tc:
        with (
            tc.tile_pool(name="persist", bufs=1) as pp,
            tc.tile_pool(name="xq", bufs=20) as xp,
            tc.tile_pool(name="es", bufs=6) as ep,
            tc.tile_pool(name="cs", bufs=2) as csp,
            tc.tile_pool(name="bc", bufs=1) as bcp,
            tc.tile_pool(name="ob", bufs=3) as obp,
            tc.tile_pool(name="s_ps", bufs=2, space="PSUM") as sps,
            tc.tile_pool(name="pv_ps", bufs=2, space="PSUM") as pvps,
        ):
            wqkv_g = pp.tile([128, 8 * 384], F32R)  # [k-part, (kt, o)]
            wo_g = pp.tile([128, 1024], BF16)
            qT = pp.tile([128, 4096], BF16)
            kTt = pp.tile([128, 4096], BF16)
            vT = pp.tile([128, 4096], BF16)
            # V with ones column: [t-part, (h, jt32, dv64|1)]
            vv = pp.tile([128, 2 * 32 * 65], BF16)
            attnT = [pp.tile([128, 2048], BF16, name=f"attnT{bb}") for bb in range(B)]
            identf = pp.tile([128, 128], F32)
            identb = pp.tile([128, 128], BF16)

            make_identity(nc, identf[:])
            ones1r = pp.tile([1, 64], F32R)
            nc.vector.tensor_copy(identb[:], identf[:])
            ones_f = pp.tile([128, 64], F32)
            nc.vector.memset(ones_f[:], 1.0)
            nc.vector.tensor_copy(ones1r[:], ones_f[0:1, :])
            vv_v = vv[:].rearrange("p (v j c) -> p v j c", v=2, c=65)
            nc.vector.tensor_copy(
                vv_v[:, :, :, 64:65],
                ones_f[:].rearrange("p (v j c) -> p v j c", v=2, c=1),
            )

            # ---------- upfront DMAs ----------
            # wqkv per-kt so the first chain starts as soon as kt=0 lands
            for kt in range(8):
                nc.sync.dma_start(
                    wqkv_g[:, kt * 384 : (kt + 1) * 384],
                    wqkvT_d[kt * 128 : (kt + 1) * 128, :],
                )
            wor = pp.tile([128, 1024], F32)
            nc.sync.dma_start(wor[:], woT_d[:])
            nc.vector.tensor_copy(wo_g[:], wor[:])

            xqs = []
            for q in range(4):
                xq = [
                    xp.tile([128, 1024], F32R, tag="xq", name=f"xq{q}_{i}")
                    for i in range(8)
                ]
                for kt in range(8):
                    nc.gpsimd.dma_start(
                        xq[kt][:],
                        xT_d[kt * 128 : (kt + 1) * 128, q * 1024 : (q + 1) * 1024],
                    )
                xqs.append(xq)

            # ---------- PE warmup: identity transposes with no DMA deps ----------
            for w in range(24):
                wt = sps.tile([128, 128], F32, tag="s", name=f"warm{w}")
                nc.tensor.transpose(wt[:], identf[:], identf[:])

            # ---------- emit helpers ----------
            evac_flip = [0]

            chain_parts = {}

            def emit_chain(q, ot, th, eng=None, part=None):
                """QKV projection chain: 512 tokens x 128 out-channels.

                part=0 emits the first 4 accumulation matmuls, part=1 the
                last 4 plus the evacuation (same psum tile across both);
                part=None emits the whole chain."""
                dest = (qT, kTt, vT)[ot]
                if part in (None, 0):
                    ps = sps.tile([128, 512], F32, tag="s", name=f"ch{q}_{ot}_{th}")
                    chain_parts[(q, ot, th)] = ps
                else:
                    ps = chain_parts.pop((q, ot, th))
                kts = range(8) if part is None else (
                    range(4) if part == 0 else range(4, 8)
                )
                for kt in kts:
                    nc.tensor.matmul(
                        ps[:],
                        wqkv_g[:, kt * 384 + ot * 128 : kt * 384 + (ot + 1) * 128],
                        xqs[q][kt][:, th * 512 : (th + 1) * 512],
                        start=(kt == 0),
                        stop=(kt == 7),
                    )
                if part == 0:
                    return
                col = q * 1024 + th * 512
                if eng is None:
                    eng = nc.vector if evac_flip[0] % 2 == 0 else nc.scalar
                    evac_flip[0] += 1
                if eng is nc.scalar:
                    nc.scalar.copy(dest[:, col : col + 512], ps[:])
                else:
                    eng.tensor_copy(dest[:, col : col + 512], ps[:])

            def emit_vtrans(jt):
                ptv = sps.tile([128, 128], BF16, tag="s", name=f"ptv{jt}")
                nc.tensor.transpose(ptv[:], vT[:, jt * 128 : (jt + 1) * 128], identb[:])
                nc.vector.tensor_copy(
                    vv_v[:, :, jt, 0:64],
                    ptv[:].rearrange("p (v c) -> p v c", v=2),
                )

            ob_flip = [0]

            def emit_po(pb, pib, tt):
                tg = pib * 8 + tt
                po = sps.tile([128, 1024], F32, tag="s", name=f"po{pb}_{pib}_{tt}")
                for oh in range(2):
                    nc.tensor.matmul(
                        po[:, oh * 512 : (oh + 1) * 512],
                        attnT[pb][:, tg * 128 : (tg + 1) * 128],
                        wo_g[:, oh * 512 : (oh + 1) * 512],
                        start=True,
                        stop=True,
                    )
                ob = obp.tile([128, 1024], BF16, tag="ob", name=f"ob{pb}_{pib}_{tt}")
                nc.vector.tensor_copy(ob[:], po[:])
                row = pb * 2048 + tg * 128
                nc.sync.dma_start(out_d[row : row + 128, :], ob[:])

            norm_state = {}

            def emit_pvs(b, ib, pv, tail=False):
                """Evacuate the pv accumulators (fast, frees psum banks).
                The colsum rows go first (they gate the lazy norm); on the
                tail the bulk copies go to ScalarE (no more exps there)."""
                tiles = []
                for h in range(2):
                    cs = csp.tile([1, 1024], F32, tag="cs", name=f"cs{b}_{ib}_{h}")
                    nc.vector.tensor_copy(cs[:], pv[h][64:65, :])
                    tiles.append(cs)
                pvs = []
                for h in range(2):
                    t = csp.tile([64, 1024], F32, tag="pvs", name=f"pvs{b}_{ib}_{h}")
                    if tail:
                        nc.scalar.copy(t[:], pv[h][0:64, :])
                    else:
                        nc.vector.tensor_copy(t[:], pv[h][0:64, :])
                    pvs.append((t, tiles[h]))
                norm_state[(b, ib)] = pvs

            def emit_norm1(b, ib):
                """Norm stage 1: colsum rows -> partition-major -> reciprocal."""
                pvs = norm_state[(b, ib)]
                ptp = sps.tile([128, 16], F32, tag="s", name=f"ptp{b}_{ib}")
                for h in range(2):
                    for blk in range(8):
                        c = blk * 2 + h
                        nc.tensor.transpose(
                            ptp[:, c : c + 1],
                            pvs[h][1][0:1, blk * 128 : (blk + 1) * 128],
                            identf[0:1, 0:1],
                        )
                rt = csp.tile([128, 16], F32, tag="rt", name=f"rt{b}_{ib}")
                nc.vector.tensor_copy(rt[:], ptp[:])
                rcp = csp.tile([128, 16], F32, tag="rcp", name=f"rcp{b}_{ib}")
                nc.vector.reciprocal(rcp[:], rt[:])
                norm_state[(b, ib, "rcp")] = rcp

            def emit_norm2(b, ib, pe_bcast=False):
                """Norm stage 2: transpose back, broadcast, apply."""
                pvs = norm_state.pop((b, ib))
                rcp = norm_state.pop((b, ib, "rcp"))
                r2s = []
                for h in range(2):
                    r2p = sps.tile([1, 1024], F32, tag="s", name=f"r2p{b}_{ib}_{h}")
                    for blk in range(8):
                        c = blk * 2 + h
                        nc.tensor.transpose(
                            r2p[0:1, blk * 128 : (blk + 1) * 128],
                            rcp[:, c : c + 1],
                            identf[:],
                        )
                    r2 = csp.tile([1, 1024], F32, tag="r2", name=f"r2{b}_{ib}_{h}")
                    nc.vector.tensor_copy(r2[:], r2p[:])
                    r2s.append(r2)
                for h in range(2):
                    if pe_bcast:
                        r2r = csp.tile(
                            [1, 1024], F32R, tag="r2r", name=f"r2r{b}_{ib}_{h}"
                        )
                        nc.vector.tensor_copy(r2r[:], r2s[h][:])
                        bcp_ps = sps.tile(
                            [64, 1024], F32, tag="s", name=f"bcp{b}_{ib}_{h}"
                        )
                        for ih in range(2):
                            nc.tensor.matmul(
                                bcp_ps[:, ih * 512 : (ih + 1) * 512],
                                ones1r[:],
                                r2r[0:1, ih * 512 : (ih + 1) * 512],
                                start=True,
                                stop=True,
                            )
                        nc.vector.tensor_tensor(
                            attnT[b][
                                h * 64 : (h + 1) * 64, ib * 1024 : (ib + 1) * 1024
                            ],
                            pvs[h][0][:],
                            bcp_ps[:],
                            mult,
                        )
                    else:
                        bc = bcp.tile([64, 1024], F32, tag="bc", name=f"bc{b}_{ib}_{h}")
                        nc.gpsimd.partition_broadcast(bc[:], r2s[h][:])
                        nc.vector.tensor_tensor(
                            attnT[b][
                                h * 64 : (h + 1) * 64, ib * 1024 : (ib + 1) * 1024
                            ],
                            pvs[h][0][:],
                            bc[:],
                            mult,
                        )

            def emit_norm(b, ib):
                emit_norm1(b, ib)
                emit_norm2(b, ib)

            # ---------- bootstrap: QKV(b0) + V(b0) transposes ----------
            for q in (0, 1):
                for ot in (1, 2, 0):  # K, V, Q order
                    for th in range(2):
                        emit_chain(q, ot, th)
            for jt in range(2):
                emit_vtrans(jt)

            # ---------- fused attention loop ----------
            # filler[block][jt] = list of callables emitted after that jt
            filler = {bi: {} for bi in range(4)}
            # block 0: QKV chains for batch 1 (K and V; Q deferred)
            b0_sched = [
                (1, 2, 0), (2, 2, 0), (1, 2, 1), (2, 2, 1),
                (1, 3, 0), (2, 3, 0), (1, 3, 1), (2, 3, 1),
            ]
            for i, (ot, q, th) in enumerate(b0_sched):
                jt = 2 * i  # halves at (jt, jt+1) covering all 16 jts
                filler[0].setdefault(jt, []).append(
                    lambda q=q, ot=ot, th=th: emit_chain(q, ot, th, part=0)
                )
                filler[0].setdefault(jt + 1, []).append(
                    lambda q=q, ot=ot, th=th: emit_chain(
                        q, ot, th, eng=nc.vector, part=1
                    )
                )
            for i in range(14):  # vtrans 2..15 spread through block 0
                filler[0].setdefault(i, []).append(lambda jt=2 + i: emit_vtrans(jt))
            # block 1: Q(b1,ib0), V(b1) transposes, norm(block0), po(b0,ib0)
            filler[1].setdefault(0, []).append(lambda: emit_chain(2, 0, 0, part=0))
            filler[1].setdefault(1, []).append(
                lambda: emit_chain(2, 0, 0, eng=nc.vector, part=1)
            )
            filler[1].setdefault(2, []).append(lambda: emit_chain(2, 0, 1, part=0))
            filler[1].setdefault(3, []).append(
                lambda: emit_chain(2, 0, 1, eng=nc.vector, part=1)
            )
            for i in range(16):
                filler[1].setdefault(2 + i // 4, []).append(
                    lambda jt=16 + i: emit_vtrans(jt)
                )
            filler[1].setdefault(5, []).append(lambda: emit_norm1(0, 0))
            filler[1].setdefault(6, []).append(lambda: emit_norm2(0, 0))
            for tt in range(8):
                filler[1].setdefault(7 + tt, []).append(
                    lambda tt=tt: emit_po(0, 0, tt)
                )
            # block 2: Q(b1,ib1), norm(block1), po(b0,ib1)
            filler[2].setdefault(0, []).append(lambda: emit_chain(3, 0, 0, part=0))
            filler[2].setdefault(1, []).append(
                lambda: emit_chain(3, 0, 0, eng=nc.vector, part=1)
            )
            filler[2].setdefault(2, []).append(lambda: emit_chain(3, 0, 1, part=0))
            filler[2].setdefault(3, []).append(
                lambda: emit_chain(3, 0, 1, eng=nc.vector, part=1)
            )
            filler[2].setdefault(4, []).append(lambda: emit_norm1(0, 1))
            filler[2].setdefault(5, []).append(lambda: emit_norm2(0, 1))
            for tt in range(8):
                filler[2].setdefault(7 + tt, []).append(
                    lambda tt=tt: emit_po(0, 1, tt)
                )
            # block 3: norm(block2), po(b1,ib0) (4 units held for the tail)
            filler[3].setdefault(1, []).append(lambda: emit_norm1(1, 0))
            filler[3].setdefault(2, []).append(lambda: emit_norm2(1, 0))
            for tt in range(4):
                filler[3].setdefault(5 + 2 * tt, []).append(
                    lambda tt=tt: emit_po(1, 0, tt)
                )

            for bi in range(4):
                b, ib = bi // 2, bi % 2
                i0 = b * 2048 + ib * 1024
                pv = [
                    pvps.tile([65, 1024], F32, tag="pv", name=f"pv{b}_{ib}_{h}")
                    for h in range(2)
                ]
                prev = None
                for jt in range(16):
                    j0 = b * 2048 + jt * 128
                    e_h = [
                        ep.tile([128, 1024], BF16, tag="e", name=f"e{b}_{ib}_{jt}_{h}")
                        for h in range(2)
                    ]
                    for h in range(2):
                        st = sps.tile(
                            [128, 1024], F32, tag="s", name=f"s{b}_{ib}_{jt}_{h}"
                        )
                        kTl = kTt[h * 64 : (h + 1) * 64, j0 : j0 + 128]
                        for ih in range(2):
                            nc.tensor.matmul(
                                st[:, ih * 512 : (ih + 1) * 512],
                                kTl,
                                qT[
                                    h * 64 : (h + 1) * 64,
                                    i0 + ih * 512 : i0 + (ih + 1) * 512,
                                ],
                                start=True,
                                stop=True,
                                tile_position=(h * 64, 0),
                            )
                        nc.scalar.activation(e_h[h][:], st[:], Exp, scale=SCALE)
                    if prev is not None:
                        jv = b * 16 + (jt - 1)
                        for h in range(2):
                            for ih in range(2):
                                nc.tensor.matmul(
                                    pv[h][:, ih * 512 : (ih + 1) * 512],
                                    vv_v[:, h, jv, :],
                                    prev[h][:, ih * 512 : (ih + 1) * 512],
                                    start=(jt - 1 == 0),
                                    stop=False,
                                )
                    prev = e_h
                    for fn in filler[bi].get(jt, ()):
                        fn()
                # flush PV for jt=15
                jv = b * 16 + 15
                for h in range(2):
                    for ih in range(2):
                        nc.tensor.matmul(
                            pv[h][:, ih * 512 : (ih + 1) * 512],
                            vv_v[:, h, jv, :],
                            prev[h][:, ih * 512 : (ih + 1) * 512],
                            start=False,
                            stop=True,
                        )
                emit_pvs(b, ib, pv, tail=(bi == 3))

            # ---------- tail: last norm overlapped with reserved po units ----------
            emit_po(1, 0, 4)
            emit_po(1, 0, 5)
            emit_norm1(1, 1)
            emit_po(1, 0, 6)
            emit_norm2(1, 1, pe_bcast=True)
            emit_po(1, 0, 7)
            for tt in range(8):
                emit_po(1, 1, tt)

    nc.compile()
    return nc


_NC = None


def _get_nc():
    global _NC
    if _NC is None:
        _NC = build_nc()
    return _NC


def _gate(mask):
    """Exact jax fp32 gate: sigmoid(m) > 0.5 (fp32 logistic rounding)."""
    mask = np.asarray(mask, dtype=np.float32)
    return (np.float32(1.0) / (np.float32(1.0) + np.exp(-mask))) > np.float32(0.5)


def make_in_maps(x, qkv_weight, qkv_weight_mask, out_weight, out_weight_mask):
    x = np.asarray(x, dtype=np.float32)
    wq = np.asarray(qkv_weight, dtype=np.float32) * _gate(qkv_weight_mask)
    wo = np.asarray(out_weight, dtype=np.float32) * _gate(out_weight_mask)

    xT = np.ascontiguousarray(x.reshape(T, DIM).T)
    in_maps = []
    for c in range(NCORES):
        r0 = c * DV
        sl = slice(r0, r0 + DV)
        w_shard = np.concatenate(
            [wq[sl], wq[DIM + r0 : DIM + r0 + DV], wq[2 * DIM + r0 : 2 * DIM + r0 + DV]],
            axis=0,
        )  # [384, 1024] rows = (q h1,h2 | k h1,h2 | v h1,h2)
        in_maps.append(
            {
                "xT": xT,
                "wqkvT": np.ascontiguousarray(w_shard.T),
                "woT": np.ascontiguousarray(wo[:, sl].T),
            }
        )
    return in_maps


LAST_RESULTS = None  # BassKernelResults of the most recent run (for profiling)


def kernel(
    x,
    qkv_weight,
    qkv_weight_mask,
    out_weight,
    out_weight_mask,
    out_bias,
    out_bias_mask,
    _trace=False,
    _tmpdir=None,
):
    global LAST_RESULTS
    from concourse.bass_utils import run_bass_kernel_spmd

    nc = _get_nc()
    in_maps = make_in_maps(x, qkv_weight, qkv_weight_mask, out_weight, out_weight_mask)
    res = run_bass_kernel_spmd(
        nc, in_maps, list(range(NCORES)), trace=_trace, tmpdir=_tmpdir
    )
    LAST_RESULTS = res
    out = np.zeros((T, DIM), dtype=np.float32)
    for r in res.results:
        out += np.asarray(r["out"]).astype(np.float32)
    out_bias = np.asarray(out_bias, dtype=np.float32)
    out += np.where(_gate(out_bias_mask), out_bias, np.float32(0.0))[None, :]
    return out.reshape(B, N, DIM)
